# revision 2
# baseline (speedup 1.0000x reference)
"""Trainium2 Bass kernel for nn_DataEmbedding (embedding_lookup).

Reference computation (B=32, L=4096, C_IN=7, D=512):
  out = value_emb + pos_emb + temp_emb
  value_emb = TokenEmbedding(x) @ proj_w.T + proj_b   (73+1 tiny conv1d's, k=8)
  pos_emb   = sinusoid_table(L, D)
  temp_emb  = sum of 4 fixed sinusoid-table lookups from x_mark (indices in [0,7))

Device algorithm (per core, 4 batches):
  * TokenEmbedding+projection collapse into ONE size-8 conv over L:
      value_emb[b,l,d] = sum_{m,c} A[d,m,c] * xpad[b, l+m, c] + proj_b[d]
    with A = einsum(proj_w[:, :511].reshape(D,73,7), kernels[:73]) + c==0 term.
  * The 4 temporal lookups are a 28-row one-hot matmul (tables only ever
    indexed at rows 0..6 where all four sinusoid tables agree).  One-hot
    rows are built on the host (pure relayout of the int indices) and
    stacked over the im2col rows -> ONE K=84 bf16 matmul per tile.
  * pos_emb + proj_b are one [L, D] bf16 table added during PSUM eviction.

Performance structure (all engines balanced under the ~62us/core DMA floor):
  * everything bf16 (tolerance is 2e-2; bf16 matmul err ~4e-3): halves
    output DMA bytes, gives 1 cyc/row matmul + fast LDWEIGHTS.
  * positions are interleaved stride-8 inside each 1024-position group:
    matmul tile t covers positions {g*1024 + 8p + t}, so PSUM partition p
    always holds 8 consecutive output rows -> every output DMA is one
    fully contiguous 1MB transfer (8KB per partition), and the pos-table
    loads are contiguous the same way.  The interleave costs nothing: the
    matmul reads the stationary operand through a stride-8 access pattern.
  * PSUM eviction is split: 5 of 8 tiles per group go to the Vector engine
    (tensor_tensor add, 1x rate from PSUM), 3 go to Scalar (activation
    copy PSUM->SBUF) + GpSimd (bf16 add in SBUF).

Sharding: pure data parallel over batch: 32 batches -> 8 cores x 4 batches.
"""

import os
import sys
import ml_dtypes
import numpy as np

for _p in ("/opt/trn_rl_repo", "/opt/pypackages"):
    if os.path.isdir(_p) and _p not in sys.path:
        sys.path.append(_p)

from contextlib import ExitStack

import concourse.bass as bass
import concourse.tile as tile
from concourse import bacc, mybir
from concourse.bass_utils import run_bass_kernel_spmd

# ---------------------------------------------------------------- constants
B, L, C_IN, D = 32, 4096, 7, 512
KS, NK, M = 8, 74, 7          # kernel_size, num_kernels, history
PROJ_IN = 73 * C_IN + 1       # 512
N_CORES = 8
NB = B // N_CORES             # batches per core = 4
R = L + KS                    # padded row length for x^T (4104)
KIM = KS * C_IN               # im2col rows = 56
KOH = 4 * 7                   # one-hot rows = 28
KTOT = KIM + KOH              # fused contraction = 84
P = 128                       # positions per matmul tile
GT = 8                        # tiles per group (position stride)
G = P * GT                    # positions per group = 1024
NG = L // G                   # groups per batch = 4

F32 = mybir.dt.float32
BF16 = mybir.dt.bfloat16

# eviction engine per tile-in-group: 'v' = Vector add, 's' = Scalar copy +
# GpSimd add.  5v/3s balances DVE (~658ns/tile) vs GpSimd (~1us/tile).
EVICT = "vsvsvsvv"


def _sinusoid_table(n, d):
    pos = np.arange(n, dtype=np.float32)[:, None]
    div = np.exp(np.arange(0, d, 2, dtype=np.float32) * (-np.log(10000.0) / d))
    tab = np.zeros((n, d), dtype=np.float32)
    tab[:, 0::2] = np.sin(pos * div)
    tab[:, 1::2] = np.cos(pos * div)
    return tab


_POS_CACHE = None


def _pos_const():
    global _POS_CACHE
    if _POS_CACHE is None:
        _POS_CACHE = _sinusoid_table(L, D)
    return _POS_CACHE


def _host_prep(x, x_mark, kernels, proj_w, proj_b):
    """Build per-core inputs. All heavy math stays on device; this is layout
    glue plus the tiny [512,511]x[73,8] weight fold."""
    x = np.asarray(x, dtype=np.float32)
    x_mark = np.asarray(x_mark)
    kernels = np.asarray(kernels, dtype=np.float32)
    proj_w = np.asarray(proj_w, dtype=np.float32)
    proj_b = np.asarray(proj_b, dtype=np.float32)

    # x^T, left-padded by M zeros along L: [B, 7, R], bf16
    xpadt = np.zeros((B, C_IN, R), dtype=ml_dtypes.bfloat16)
    xpadt[:, :, M : M + L] = x.transpose(0, 2, 1).astype(ml_dtypes.bfloat16)

    # one-hot of x_mark (pure relayout of the int indices): [B, 28, L] bf16,
    # row 7j+v = (x_mark[:, :, j] == v)
    xm = x_mark.astype(np.int64)
    oh = (xm[:, :, :, None] == np.arange(7)[None, None, None, :])  # [B,L,4,7]
    onehot = np.ascontiguousarray(
        oh.transpose(0, 2, 3, 1).reshape(B, KOH, L).astype(ml_dtypes.bfloat16)
    )

    # fused conv weight A[d, m, c]
    p3 = proj_w[:, : 73 * C_IN].reshape(D, 73, C_IN)
    A = np.einsum("dkc,km->dmc", p3, kernels[:73], dtype=np.float32)
    A[:, :, 0] += np.outer(proj_w[:, 511], kernels[73])
    w_pack = A.transpose(1, 2, 0).reshape(KIM, D)  # row 7m+c

    # temporal tables: all four sinusoid tables agree on rows 0..6.
    tab7 = _sinusoid_table(7, D)  # [7, D]
    wtab = np.concatenate([np.tile(tab7, (4, 1)), w_pack], axis=0)  # [84, D]
    wtab = np.ascontiguousarray(wtab.astype(ml_dtypes.bfloat16))

    # positional + bias table (bf16: |values| <= ~1, rounding ~2e-3 abs,
    # negligible vs output scale ~22)
    posb = np.ascontiguousarray(
        (_pos_const() + proj_b[None, :]).astype(ml_dtypes.bfloat16)
    )

    in_maps = []
    for core in range(N_CORES):
        sl = slice(core * NB, (core + 1) * NB)
        in_maps.append(
            {
                "xpadt": np.ascontiguousarray(xpadt[sl]),
                "onehot": np.ascontiguousarray(onehot[sl]),
                "wtab": wtab,
                "posb": posb,
            }
        )
    return in_maps


# ---------------------------------------------------------------- bass build
def build_nc(evict=EVICT, psum_bufs=8, stage_bufs=3):
    nc = bacc.Bacc("TRN2", target_bir_lowering=False, debug=False)

    xpadt_d = nc.dram_tensor("xpadt", (NB, C_IN, R), BF16, kind="ExternalInput")
    onehot_d = nc.dram_tensor("onehot", (NB, KOH, L), BF16, kind="ExternalInput")
    wtab_d = nc.dram_tensor("wtab", (KTOT, D), BF16, kind="ExternalInput")
    posb_d = nc.dram_tensor("posb", (L, D), BF16, kind="ExternalInput")
    out_d = nc.dram_tensor("out", (NB, L, D), BF16, kind="ExternalOutput")

    with tile.TileContext(nc) as tc, ExitStack() as ctx:
        dma = nc.sync
        consts = ctx.enter_context(tc.tile_pool(name="consts", bufs=1))
        lhs_pool = ctx.enter_context(tc.tile_pool(name="lhs", bufs=2))
        xt_pool = ctx.enter_context(tc.tile_pool(name="xt", bufs=2))
        pos_pool = ctx.enter_context(tc.tile_pool(name="pos", bufs=1))
        stage_pool = ctx.enter_context(tc.tile_pool(name="stage", bufs=stage_bufs))
        psum_pool = ctx.enter_context(
            tc.tile_pool(name="psum", bufs=psum_bufs, space="PSUM")
        )

        wtab_s = consts.tile([KTOT, D], BF16, tag="wtab")
        dma.dma_start(wtab_s[:], wtab_d.ap())

        # positional(+bias) table, SBUF-resident: NG tiles of [128, GT*D].
        # pos_tiles[g][p, t*D:(t+1)*D] = posb[g*G + 8p + t, :]  (stride-8
        # position interleave) -> each partition reads 8 consecutive rows,
        # i.e. one contiguous 8KB chunk.
        pos_tiles = []
        for g in range(NG):
            pt = pos_pool.tile([P, GT * D], BF16, tag=f"pos{g}")
            src = posb_d.ap()[g * G : (g + 1) * G, :]
            src = src.rearrange("(p t) d -> p t d", p=P)
            dma.dma_start(pt[:].rearrange("p (t d) -> p t d", d=D), src)
            pos_tiles.append(pt)

        def build_lhs(b):
            """Fused stationary operand for batch b: rows 0..27 one-hot
            (HBM direct), rows 28..83 im2col of x^T (8 shifted SBUF->SBUF
            copies of one HBM read)."""
            xt = xt_pool.tile([C_IN, R], BF16, tag="xt")
            dma.dma_start(xt[:], xpadt_d.ap()[b])
            lhs = lhs_pool.tile([KTOT, L], BF16, tag="lhs")
            dma.dma_start(lhs[0:KOH, :], onehot_d.ap()[b])
            for m in range(KS):
                dma.dma_start(
                    lhs[KOH + C_IN * m : KOH + C_IN * (m + 1), :],
                    xt[:, m : m + L],
                )
            return lhs

        lhs = build_lhs(0)
        for b in range(NB):
            next_lhs = build_lhs(b + 1) if b + 1 < NB else None
            for g in range(NG):
                # [84, GT, P] view: index [k, t, p] = column g*G + 8p + t
                lhs_g = lhs[:, g * G : (g + 1) * G].rearrange(
                    "k (p s) -> k s p", s=GT
                )
                stage = stage_pool.tile([P, GT * D], BF16, tag="stage")
                pos_g = pos_tiles[g]
                for t in range(GT):
                    ps = psum_pool.tile([P, D], F32, tag="ps")
                    nc.tensor.matmul(
                        ps[:],
                        lhs_g[:, t, :],
                        wtab_s[:],
                        start=True,
                        stop=True,
                    )
                    dsl = slice(D * t, D * (t + 1))
                    if evict[t] == "v":
                        nc.vector.tensor_tensor(
                            out=stage[:, dsl],
                            in0=ps[:],
                            in1=pos_g[:, dsl],
                            op=mybir.AluOpType.add,
                        )
                    else:
                        nc.scalar.copy(stage[:, dsl], ps[:])
                        nc.gpsimd.tensor_tensor(
                            out=stage[:, dsl],
                            in0=stage[:, dsl],
                            in1=pos_g[:, dsl],
                            op=mybir.AluOpType.add,
                        )
                # partition p holds output rows g*G + 8p .. g*G + 8p + 7:
                # one fully contiguous 1MB transfer (8KB per partition).
                dst = out_d.ap()[b, g * G : (g + 1) * G, :]
                dst = dst.rearrange("(p t) d -> p t d", p=P)
                dma.dma_start(dst, stage[:].rearrange("p (t d) -> p t d", d=D))
            lhs = next_lhs

    nc.compile()
    return nc


_NC_CACHE = None


def _get_nc():
    global _NC_CACHE
    if _NC_CACHE is None:
        _NC_CACHE = build_nc()
    return _NC_CACHE


TRACE = False          # set by test.py to capture an NTFF profile
LAST_RESULT = None     # BassKernelResults of the most recent run


def _run_once(in_maps):
    global LAST_RESULT
    nc = _get_nc()
    res = run_bass_kernel_spmd(
        nc, in_maps, core_ids=list(range(N_CORES)), trace=TRACE
    )
    LAST_RESULT = res
    return np.concatenate(
        [np.asarray(r["out"], dtype=np.float32) for r in res.results], axis=0
    )


def _run_subprocess(inputs):
    """Crash-isolated fallback: run in a fresh interpreter (a device fault can
    wedge the parent process's jax runtime)."""
    import pickle
    import subprocess
    import tempfile

    with tempfile.TemporaryDirectory() as td:
        fin = os.path.join(td, "in.pkl")
        fout = os.path.join(td, "out.npy")
        with open(fin, "wb") as f:
            pickle.dump(inputs, f)
        code = (
            "import pickle, numpy as np, sys;"
            f"sys.path.insert(0, {os.path.dirname(os.path.abspath(__file__))!r});"
            "import kernel as K;"
            f"ins = pickle.load(open({fin!r}, 'rb'));"
            "out = K._run_once(K._host_prep(**ins));"
            f"np.save({fout!r}, out)"
        )
        subprocess.run([sys.executable, "-c", code], check=True, timeout=1800)
        return np.load(fout)


def kernel(x, x_mark, kernels, proj_w, proj_b):
    inputs = dict(x=x, x_mark=x_mark, kernels=kernels, proj_w=proj_w,
                  proj_b=proj_b)
    in_maps = _host_prep(**inputs)
    # the TRN fleet shows rare transient NRT_EXEC_UNIT_UNRECOVERABLE faults;
    # retry in-process first, then in fresh subprocesses.
    for attempt in range(2):
        try:
            return _run_once(in_maps)
        except Exception:
            pass
    for attempt in range(3):
        try:
            return _run_subprocess(inputs)
        except Exception:
            if attempt == 2:
                raise
    raise RuntimeError("unreachable")


# revision 3
# speedup vs baseline: 1.0478x; 1.0478x over previous
"""Trainium2 Bass kernel for nn_DataEmbedding (embedding_lookup).

Reference computation (B=32, L=4096, C_IN=7, D=512):
  out = value_emb + pos_emb + temp_emb
  value_emb = TokenEmbedding(x) @ proj_w.T + proj_b   (73+1 tiny conv1d's, k=8)
  pos_emb   = sinusoid_table(L, D)
  temp_emb  = sum of 4 fixed sinusoid-table lookups from x_mark (indices in [0,7))

Device algorithm (per core, 4 batches):
  * TokenEmbedding+projection collapse into ONE size-8 conv over L:
      value_emb[b,l,d] = sum_{m,c} A[d,m,c] * xpad[b, l+m, c] + proj_b[d]
    with A = einsum(proj_w[:, :511].reshape(D,73,7), kernels[:73]) + c==0 term.
  * The 4 temporal lookups are a 28-row one-hot matmul (tables only ever
    indexed at rows 0..6 where all four sinusoid tables agree).  One-hot
    rows are built on the host (pure relayout of the int indices) and
    stacked over the im2col rows -> ONE K=84 bf16 matmul per tile.
  * pos_emb + proj_b are one [L, D] bf16 table added during PSUM eviction.

Performance structure (all engines balanced under the ~62us/core DMA floor):
  * everything bf16 (tolerance is 2e-2; bf16 matmul err ~4e-3): halves
    output DMA bytes, gives 1 cyc/row matmul + fast LDWEIGHTS.
  * positions are interleaved stride-8 inside each 1024-position group:
    matmul tile t covers positions {g*1024 + 8p + t}, so PSUM partition p
    always holds 8 consecutive output rows -> every output DMA is one
    fully contiguous 1MB transfer (8KB per partition), and the pos-table
    loads are contiguous the same way.  The interleave costs nothing: the
    matmul reads the stationary operand through a stride-8 access pattern.
  * PSUM eviction is split: 5 of 8 tiles per group go to the Vector engine
    (tensor_tensor add, 1x rate from PSUM), 3 go to Scalar (activation
    copy PSUM->SBUF) + GpSimd (bf16 add in SBUF).

Sharding: pure data parallel over batch: 32 batches -> 8 cores x 4 batches.
"""

import os
import sys
import ml_dtypes
import numpy as np

for _p in ("/opt/trn_rl_repo", "/opt/pypackages"):
    if os.path.isdir(_p) and _p not in sys.path:
        sys.path.append(_p)

from contextlib import ExitStack

import concourse.bass as bass
import concourse.tile as tile
from concourse import bacc, mybir
from concourse.bass_utils import run_bass_kernel_spmd

# ---------------------------------------------------------------- constants
B, L, C_IN, D = 32, 4096, 7, 512
KS, NK, M = 8, 74, 7          # kernel_size, num_kernels, history
PROJ_IN = 73 * C_IN + 1       # 512
N_CORES = 8
NB = B // N_CORES             # batches per core = 4
R = L + KS                    # padded row length for x^T (4104)
KIM = KS * C_IN               # im2col rows = 56
KOH = 4 * 7                   # one-hot rows = 28
KTOT = KIM + KOH              # fused contraction = 84
P = 128                       # positions per matmul tile
GT = 8                        # tiles per group (position stride)
G = P * GT                    # positions per group = 1024
NG = L // G                   # groups per batch = 4

F32 = mybir.dt.float32
BF16 = mybir.dt.bfloat16

# eviction engine per tile-in-group: 'v' = Vector add, 's' = Scalar copy +
# GpSimd add.  5v/3s balances DVE (~658ns/tile) vs GpSimd (~1us/tile).
EVICT = "vsvsvsvv"


def _sinusoid_table(n, d):
    pos = np.arange(n, dtype=np.float32)[:, None]
    div = np.exp(np.arange(0, d, 2, dtype=np.float32) * (-np.log(10000.0) / d))
    tab = np.zeros((n, d), dtype=np.float32)
    tab[:, 0::2] = np.sin(pos * div)
    tab[:, 1::2] = np.cos(pos * div)
    return tab


_POS_CACHE = None


def _pos_const():
    global _POS_CACHE
    if _POS_CACHE is None:
        _POS_CACHE = _sinusoid_table(L, D)
    return _POS_CACHE


def _host_prep(x, x_mark, kernels, proj_w, proj_b):
    """Build per-core inputs. All heavy math stays on device; this is layout
    glue plus the tiny [512,511]x[73,8] weight fold."""
    x = np.asarray(x, dtype=np.float32)
    x_mark = np.asarray(x_mark)
    kernels = np.asarray(kernels, dtype=np.float32)
    proj_w = np.asarray(proj_w, dtype=np.float32)
    proj_b = np.asarray(proj_b, dtype=np.float32)

    # x^T, left-padded by M zeros along L: [B, 7, R], bf16
    xpadt = np.zeros((B, C_IN, R), dtype=ml_dtypes.bfloat16)
    xpadt[:, :, M : M + L] = x.transpose(0, 2, 1).astype(ml_dtypes.bfloat16)

    # one-hot of x_mark (pure relayout of the int indices): [B, 28, L] bf16,
    # row 7j+v = (x_mark[:, :, j] == v)
    xm = x_mark.astype(np.int64)
    oh = (xm[:, :, :, None] == np.arange(7)[None, None, None, :])  # [B,L,4,7]
    onehot = np.ascontiguousarray(
        oh.transpose(0, 2, 3, 1).reshape(B, KOH, L).astype(ml_dtypes.bfloat16)
    )

    # fused conv weight A[d, m, c]
    p3 = proj_w[:, : 73 * C_IN].reshape(D, 73, C_IN)
    A = np.einsum("dkc,km->dmc", p3, kernels[:73], dtype=np.float32)
    A[:, :, 0] += np.outer(proj_w[:, 511], kernels[73])
    w_pack = A.transpose(1, 2, 0).reshape(KIM, D)  # row 7m+c

    # temporal tables: all four sinusoid tables agree on rows 0..6.
    tab7 = _sinusoid_table(7, D)  # [7, D]
    wtab = np.concatenate([np.tile(tab7, (4, 1)), w_pack], axis=0)  # [84, D]
    wtab = np.ascontiguousarray(wtab.astype(ml_dtypes.bfloat16))

    # positional + bias table (bf16: |values| <= ~1, rounding ~2e-3 abs,
    # negligible vs output scale ~22)
    posb = np.ascontiguousarray(
        (_pos_const() + proj_b[None, :]).astype(ml_dtypes.bfloat16)
    )

    in_maps = []
    for core in range(N_CORES):
        sl = slice(core * NB, (core + 1) * NB)
        in_maps.append(
            {
                "xpadt": np.ascontiguousarray(xpadt[sl]),
                "onehot": np.ascontiguousarray(onehot[sl]),
                "wtab": wtab,
                "posb": posb,
            }
        )
    return in_maps


# ---------------------------------------------------------------- bass build
def build_nc(evict=EVICT, psum_bufs=8, stage_bufs=4):
    nc = bacc.Bacc("TRN2", target_bir_lowering=False, debug=False)

    xpadt_d = nc.dram_tensor("xpadt", (NB, C_IN, R), BF16, kind="ExternalInput")
    onehot_d = nc.dram_tensor("onehot", (NB, KOH, L), BF16, kind="ExternalInput")
    wtab_d = nc.dram_tensor("wtab", (KTOT, D), BF16, kind="ExternalInput")
    posb_d = nc.dram_tensor("posb", (L, D), BF16, kind="ExternalInput")
    out_d = nc.dram_tensor("out", (NB, L, D), BF16, kind="ExternalOutput")

    with tile.TileContext(nc) as tc, ExitStack() as ctx:
        dma = nc.sync        # input DMAs: SP HWDGE ring
        odma = nc.scalar     # output DMAs: Act HWDGE ring (parallel dispatch)
        consts = ctx.enter_context(tc.tile_pool(name="consts", bufs=1))
        lhs_pool = ctx.enter_context(tc.tile_pool(name="lhs", bufs=2))
        xt_pool = ctx.enter_context(tc.tile_pool(name="xt", bufs=2))
        pos_pool = ctx.enter_context(tc.tile_pool(name="pos", bufs=1))
        stage_pool = ctx.enter_context(tc.tile_pool(name="stage", bufs=stage_bufs))
        psum_pool = ctx.enter_context(
            tc.tile_pool(name="psum", bufs=psum_bufs, space="PSUM")
        )

        def build_lhs(b):
            """Fused stationary operand for batch b: rows 0..27 one-hot
            (HBM direct), rows 28..83 im2col of x^T (8 shifted SBUF->SBUF
            copies of one HBM read)."""
            xt = xt_pool.tile([C_IN, R], BF16, tag="xt")
            dma.dma_start(xt[:], xpadt_d.ap()[b])
            lhs = lhs_pool.tile([KTOT, L], BF16, tag="lhs")
            dma.dma_start(lhs[0:KOH, :], onehot_d.ap()[b])
            for m in range(KS):
                dma.dma_start(
                    lhs[KOH + C_IN * m : KOH + C_IN * (m + 1), :],
                    xt[:, m : m + L],
                )
            return lhs

        # batch-0 operand first: the PE can start ~8us in instead of
        # waiting behind the 4MB pos-table load.
        lhs = build_lhs(0)

        wtab_s = consts.tile([KTOT, D], BF16, tag="wtab")
        dma.dma_start(wtab_s[:], wtab_d.ap())

        # positional(+bias) table, SBUF-resident: NG tiles of [128, GT*D].
        # pos_tiles[g][p, t*D:(t+1)*D] = posb[g*G + 8p + t, :]  (stride-8
        # position interleave) -> each partition reads 8 consecutive rows,
        # i.e. one contiguous 8KB chunk (flat 2D AP -> 128 descriptors).
        pos_tiles = []
        for g in range(NG):
            pt = pos_pool.tile([P, GT * D], BF16, tag=f"pos{g}")
            src = posb_d.ap()[g * G : (g + 1) * G, :]
            src = src.rearrange("(p t) d -> p (t d)", p=P)
            dma.dma_start(pt[:], src)
            pos_tiles.append(pt)

        for b in range(NB):
            next_lhs = build_lhs(b + 1) if b + 1 < NB else None
            for g in range(NG):
                # [84, GT, P] view: index [k, t, p] = column g*G + 8p + t
                lhs_g = lhs[:, g * G : (g + 1) * G].rearrange(
                    "k (p s) -> k s p", s=GT
                )
                stage = stage_pool.tile([P, GT * D], BF16, tag="stage")
                pos_g = pos_tiles[g]
                for t in range(GT):
                    ps = psum_pool.tile([P, D], F32, tag="ps")
                    nc.tensor.matmul(
                        ps[:],
                        lhs_g[:, t, :],
                        wtab_s[:],
                        start=True,
                        stop=True,
                    )
                    dsl = slice(D * t, D * (t + 1))
                    if evict[t] == "v":
                        nc.vector.tensor_tensor(
                            out=stage[:, dsl],
                            in0=ps[:],
                            in1=pos_g[:, dsl],
                            op=mybir.AluOpType.add,
                        )
                    else:
                        nc.scalar.copy(stage[:, dsl], ps[:])
                        nc.gpsimd.tensor_tensor(
                            out=stage[:, dsl],
                            in0=stage[:, dsl],
                            in1=pos_g[:, dsl],
                            op=mybir.AluOpType.add,
                        )
                # partition p holds output rows g*G + 8p .. g*G + 8p + 7:
                # one fully contiguous 1MB transfer, 128 x 8KB descriptors.
                dst = out_d.ap()[b, g * G : (g + 1) * G, :]
                dst = dst.rearrange("(p t) d -> p (t d)", p=P)
                odma.dma_start(dst, stage[:])
            lhs = next_lhs

    nc.compile()
    return nc


_NC_CACHE = None


def _get_nc():
    global _NC_CACHE
    if _NC_CACHE is None:
        _NC_CACHE = build_nc()
    return _NC_CACHE


TRACE = False          # set by test.py to capture an NTFF profile
LAST_RESULT = None     # BassKernelResults of the most recent run


def _run_once(in_maps):
    global LAST_RESULT
    nc = _get_nc()
    res = run_bass_kernel_spmd(
        nc, in_maps, core_ids=list(range(N_CORES)), trace=TRACE
    )
    LAST_RESULT = res
    return np.concatenate(
        [np.asarray(r["out"], dtype=np.float32) for r in res.results], axis=0
    )


def _run_subprocess(inputs):
    """Crash-isolated fallback: run in a fresh interpreter (a device fault can
    wedge the parent process's jax runtime)."""
    import pickle
    import subprocess
    import tempfile

    with tempfile.TemporaryDirectory() as td:
        fin = os.path.join(td, "in.pkl")
        fout = os.path.join(td, "out.npy")
        with open(fin, "wb") as f:
            pickle.dump(inputs, f)
        code = (
            "import pickle, numpy as np, sys;"
            f"sys.path.insert(0, {os.path.dirname(os.path.abspath(__file__))!r});"
            "import kernel as K;"
            f"ins = pickle.load(open({fin!r}, 'rb'));"
            "out = K._run_once(K._host_prep(**ins));"
            f"np.save({fout!r}, out)"
        )
        subprocess.run([sys.executable, "-c", code], check=True, timeout=1800)
        return np.load(fout)


def kernel(x, x_mark, kernels, proj_w, proj_b):
    inputs = dict(x=x, x_mark=x_mark, kernels=kernels, proj_w=proj_w,
                  proj_b=proj_b)
    in_maps = _host_prep(**inputs)
    # the TRN fleet shows rare transient NRT_EXEC_UNIT_UNRECOVERABLE faults;
    # retry in-process first, then in fresh subprocesses.
    for attempt in range(2):
        try:
            return _run_once(in_maps)
        except Exception:
            pass
    for attempt in range(3):
        try:
            return _run_subprocess(inputs)
        except Exception:
            if attempt == 2:
                raise
    raise RuntimeError("unreachable")


# revision 4
# speedup vs baseline: 1.2001x; 1.1453x over previous
"""Trainium2 Bass kernel for nn_DataEmbedding (embedding_lookup).

Reference computation (B=32, L=4096, C_IN=7, D=512):
  out = value_emb + pos_emb + temp_emb
  value_emb = TokenEmbedding(x) @ proj_w.T + proj_b   (73+1 tiny conv1d's, k=8)
  pos_emb   = sinusoid_table(L, D)
  temp_emb  = sum of 4 fixed sinusoid-table lookups from x_mark (indices in [0,7))

Device algorithm (per core, 4 batches):
  * TokenEmbedding+projection collapse into ONE size-8 conv over L:
      value_emb[b,l,d] = sum_{m,c} A[d,m,c] * xpad[b, l+m, c] + proj_b[d]
    with A = einsum(proj_w[:, :511].reshape(D,73,7), kernels[:73]) + c==0 term.
  * The 4 temporal lookups are a 28-row one-hot matmul (tables only ever
    indexed at rows 0..6 where all four sinusoid tables agree).
  * The whole stationary operand (one-hot rows stacked over im2col rows)
    is built host-side — a pure relayout of x / x_mark — so each batch
    needs exactly ONE input DMA.  One K=84 bf16 matmul per 128-position
    tile does all the math; pos_emb + proj_b are one [L, D] bf16 table
    added during PSUM eviction.

Performance structure (PE-bound: LDW+MM pairs measure 677ns sustained
on this part, 128 pairs ~= 87us/core floor; everything else sits under):
  * all-bf16 (tolerance 2e-2, actual err ~6e-3).
  * positions interleaved stride-8 within each 1024-position group (the
    interleave is applied host-side so device APs are contiguous):
    PSUM partition p always holds 8 consecutive output rows -> every
    output DMA is one fully contiguous 1MB transfer (128 x 8KB), pos
    loads likewise.
  * PSUM eviction at 2-bank granularity (FD=1024) to amortize per-op
    overhead and halve semaphore traffic: per 4 pairs, 2 go DVE-direct
    (tensor_tensor add from PSUM), 2 go ScE-copy + DVE bf16 add (2x).
  * output DMAs dispatch on the Act HWDGE ring, inputs on SP, so input
    dispatches never queue behind 1MB output transfers.

Sharding: pure data parallel over batch: 32 batches -> 8 cores x 4 batches.
"""

import os
import sys
import ml_dtypes
import numpy as np

for _p in ("/opt/trn_rl_repo", "/opt/pypackages"):
    if os.path.isdir(_p) and _p not in sys.path:
        sys.path.append(_p)

from contextlib import ExitStack

import concourse.bass as bass
import concourse.tile as tile
from concourse import bacc, mybir
from concourse.bass_utils import run_bass_kernel_spmd

# ---------------------------------------------------------------- constants
B, L, C_IN, D = 32, 4096, 7, 512
KS, NK, M = 8, 74, 7          # kernel_size, num_kernels, history
PROJ_IN = 73 * C_IN + 1       # 512
N_CORES = 8
NB = B // N_CORES             # batches per core = 4
KIM = KS * C_IN               # im2col rows = 56
KOH = 4 * 7                   # one-hot rows = 28
KTOT = KIM + KOH              # fused contraction = 84
P = 128                       # positions per matmul tile
GT = 8                        # tiles per group (position stride)
G = P * GT                    # positions per group = 1024
NG = L // G                   # groups per batch = 4
PAIR = 2 * D                  # eviction unit: 2 PSUM banks = 1024

F32 = mybir.dt.float32
BF16 = mybir.dt.bfloat16


def _sinusoid_table(n, d):
    pos = np.arange(n, dtype=np.float32)[:, None]
    div = np.exp(np.arange(0, d, 2, dtype=np.float32) * (-np.log(10000.0) / d))
    tab = np.zeros((n, d), dtype=np.float32)
    tab[:, 0::2] = np.sin(pos * div)
    tab[:, 1::2] = np.cos(pos * div)
    return tab


_POS_CACHE = None


def _pos_const():
    global _POS_CACHE
    if _POS_CACHE is None:
        _POS_CACHE = _sinusoid_table(L, D)
    return _POS_CACHE


# column permutation: device matmul tile (g, t) takes columns
# [ (g*GT + t)*P : +P ] of the permuted operand; column (g,t,p) must hold
# position g*G + 8p + t so that PSUM partition p = output row g*G+8p+t.
_PERM = None


def _col_perm():
    global _PERM
    if _PERM is None:
        l = np.arange(L)
        g, r = l // G, l % G
        t, p = r // P, r % P
        _PERM = (g * G + p * GT + t).astype(np.int64)  # perm[j'] = source pos
    return _PERM


def _host_prep(x, x_mark, kernels, proj_w, proj_b):
    """Build per-core inputs. All heavy math stays on device; this is layout
    glue plus the tiny [512,511]x[73,8] weight fold."""
    x = np.asarray(x, dtype=np.float32)
    x_mark = np.asarray(x_mark)
    kernels = np.asarray(kernels, dtype=np.float32)
    proj_w = np.asarray(proj_w, dtype=np.float32)
    proj_b = np.asarray(proj_b, dtype=np.float32)

    # full stationary operand [B, 84, L] bf16 (pure relayout of x/x_mark):
    #   rows 0..27   one-hot: row 7j+v = (x_mark[:, :, j] == v)
    #   rows 28..83  im2col:  row 28+7m+c = xpad[:, c, l+m]
    lhs = np.empty((B, KTOT, L), dtype=ml_dtypes.bfloat16)
    xm = x_mark.astype(np.int64)
    oh = xm[:, :, :, None] == np.arange(7)[None, None, None, :]   # [B,L,4,7]
    lhs[:, :KOH, :] = oh.transpose(0, 2, 3, 1).reshape(B, KOH, L)
    xpad = np.zeros((B, C_IN, L + KS), dtype=np.float32)
    xpad[:, :, M : M + L] = x.transpose(0, 2, 1)
    for m in range(KS):
        lhs[:, KOH + C_IN * m : KOH + C_IN * (m + 1), :] = xpad[
            :, :, m : m + L
        ]
    # stride-8 position interleave (see _col_perm)
    lhs = np.ascontiguousarray(lhs[:, :, _col_perm()])

    # fused conv weight A[d, m, c]
    p3 = proj_w[:, : 73 * C_IN].reshape(D, 73, C_IN)
    A = np.einsum("dkc,km->dmc", p3, kernels[:73], dtype=np.float32)
    A[:, :, 0] += np.outer(proj_w[:, 511], kernels[73])
    w_pack = A.transpose(1, 2, 0).reshape(KIM, D)  # row 7m+c

    # temporal tables: all four sinusoid tables agree on rows 0..6.
    tab7 = _sinusoid_table(7, D)  # [7, D]
    wtab = np.concatenate([np.tile(tab7, (4, 1)), w_pack], axis=0)  # [84, D]
    wtab = np.ascontiguousarray(wtab.astype(ml_dtypes.bfloat16))

    # positional + bias table (bf16: |values| <= ~1, rounding ~2e-3 abs,
    # negligible vs output scale ~22), rows in interleaved order so the
    # SBUF tile [128, NG*GT*D] has partition p = rows {g*G+8p+t}.
    posb = (_pos_const() + proj_b[None, :]).astype(ml_dtypes.bfloat16)
    # row r of interleaved table = position g*G + 8p + t where the SBUF
    # flat index is ((p * NG) + g) * GT + t ... simpler: build per-partition
    # layout directly: part p, free [g, t, d] = posb[g*G + 8p + t, d]
    pos_il = posb.reshape(NG, P, GT, D)            # [g, p, t, d]
    pos_il = np.ascontiguousarray(
        pos_il.transpose(1, 0, 2, 3).reshape(P, NG * GT * D)
    )  # [p, (g t d)]

    in_maps = []
    for core in range(N_CORES):
        sl = slice(core * NB, (core + 1) * NB)
        in_maps.append(
            {
                "lhs": np.ascontiguousarray(lhs[sl]),
                "wtab": wtab,
                "posil": pos_il,
            }
        )
    return in_maps


# ---------------------------------------------------------------- bass build
def build_nc(stage_bufs=4):
    nc = bacc.Bacc("TRN2", target_bir_lowering=False, debug=False)

    lhs_d = nc.dram_tensor("lhs", (NB, KTOT, L), BF16, kind="ExternalInput")
    wtab_d = nc.dram_tensor("wtab", (KTOT, D), BF16, kind="ExternalInput")
    posil_d = nc.dram_tensor("posil", (P, NG * GT * D), BF16,
                             kind="ExternalInput")
    out_d = nc.dram_tensor("out", (NB, L, D), BF16, kind="ExternalOutput")

    with tile.TileContext(nc) as tc, ExitStack() as ctx:
        dma = nc.sync        # input DMAs: SP HWDGE ring
        odma = nc.scalar     # output DMAs: Act HWDGE ring
        consts = ctx.enter_context(tc.tile_pool(name="consts", bufs=1))
        lhs_pool = ctx.enter_context(tc.tile_pool(name="lhsp", bufs=2))
        stage_pool = ctx.enter_context(tc.tile_pool(name="stage", bufs=stage_bufs))
        psum_pool = ctx.enter_context(
            tc.tile_pool(name="psum", bufs=4, space="PSUM")
        )

        # batch-0 operand first so the PE starts as early as possible
        lhs0 = lhs_pool.tile([KTOT, L], BF16, tag="lhs", name="lhs0")
        dma.dma_start(lhs0[:], lhs_d.ap()[0])

        wtab_s = consts.tile([KTOT, D], BF16, tag="wtab")
        dma.dma_start(wtab_s[:], wtab_d.ap())

        # positional(+bias) table, one DMA, SBUF-resident [128, NG*GT*D]
        pos_s = consts.tile([P, NG * GT * D], BF16, tag="pos")
        dma.dma_start(pos_s[:], posil_d.ap())

        lhs = lhs0
        for b in range(NB):
            if b + 1 < NB:
                next_lhs = lhs_pool.tile([KTOT, L], BF16, tag="lhs",
                                         name=f"lhs{b + 1}")
                dma.dma_start(next_lhs[:], lhs_d.ap()[b + 1])
            else:
                next_lhs = None
            for g in range(NG):
                stage = stage_pool.tile([P, GT * D], BF16, tag="stage")
                for j in range(4):          # 4 pair-units of 2 tiles
                    ps = psum_pool.tile([P, PAIR], F32, tag="ps")
                    for h in range(2):
                        t = 2 * j + h
                        nc.tensor.matmul(
                            ps[:, D * h : D * (h + 1)],
                            lhs[:, (g * GT + t) * P : (g * GT + t + 1) * P],
                            wtab_s[:],
                            start=True,
                            stop=True,
                        )
                    ssl = slice(PAIR * j, PAIR * (j + 1))
                    psl = slice((g * GT + 2 * j) * D, (g * GT + 2 * j + 2) * D)
                    if j % 2 == 0:
                        # DVE: add pos straight out of PSUM (1x, FD=1024)
                        nc.vector.tensor_tensor(
                            out=stage[:, ssl],
                            in0=ps[:],
                            in1=pos_s[:, psl],
                            op=mybir.AluOpType.add,
                        )
                    else:
                        # ScE copy PSUM->SBUF, then DVE bf16 add (2x)
                        nc.scalar.copy(stage[:, ssl], ps[:])
                        nc.vector.tensor_tensor(
                            out=stage[:, ssl],
                            in0=stage[:, ssl],
                            in1=pos_s[:, psl],
                            op=mybir.AluOpType.add,
                        )
                # partition p holds output rows g*G + 8p .. g*G + 8p + 7:
                # one fully contiguous 1MB transfer, 128 x 8KB descriptors.
                dst = out_d.ap()[b, g * G : (g + 1) * G, :]
                dst = dst.rearrange("(p t) d -> p (t d)", p=P)
                odma.dma_start(dst, stage[:])
            lhs = next_lhs

    nc.compile()
    return nc


_NC_CACHE = None


def _get_nc():
    global _NC_CACHE
    if _NC_CACHE is None:
        _NC_CACHE = build_nc()
    return _NC_CACHE


TRACE = False          # set by test.py to capture an NTFF profile
LAST_RESULT = None     # BassKernelResults of the most recent run


def _run_once(in_maps):
    global LAST_RESULT
    nc = _get_nc()
    res = run_bass_kernel_spmd(
        nc, in_maps, core_ids=list(range(N_CORES)), trace=TRACE
    )
    LAST_RESULT = res
    return np.concatenate(
        [np.asarray(r["out"], dtype=np.float32) for r in res.results], axis=0
    )


def _run_subprocess(inputs):
    """Crash-isolated fallback: run in a fresh interpreter (a device fault can
    wedge the parent process's jax runtime)."""
    import pickle
    import subprocess
    import tempfile

    with tempfile.TemporaryDirectory() as td:
        fin = os.path.join(td, "in.pkl")
        fout = os.path.join(td, "out.npy")
        with open(fin, "wb") as f:
            pickle.dump(inputs, f)
        code = (
            "import pickle, numpy as np, sys;"
            f"sys.path.insert(0, {os.path.dirname(os.path.abspath(__file__))!r});"
            "import kernel as K;"
            f"ins = pickle.load(open({fin!r}, 'rb'));"
            "out = K._run_once(K._host_prep(**ins));"
            f"np.save({fout!r}, out)"
        )
        subprocess.run([sys.executable, "-c", code], check=True, timeout=1800)
        return np.load(fout)


def kernel(x, x_mark, kernels, proj_w, proj_b):
    inputs = dict(x=x, x_mark=x_mark, kernels=kernels, proj_w=proj_w,
                  proj_b=proj_b)
    in_maps = _host_prep(**inputs)
    # the TRN fleet shows rare transient NRT_EXEC_UNIT_UNRECOVERABLE faults;
    # retry in-process first, then in fresh subprocesses.
    for attempt in range(2):
        try:
            return _run_once(in_maps)
        except Exception:
            pass
    for attempt in range(3):
        try:
            return _run_subprocess(inputs)
        except Exception:
            if attempt == 2:
                raise
    raise RuntimeError("unreachable")


# revision 7
# speedup vs baseline: 1.3844x; 1.1536x over previous
"""Trainium2 Bass kernel for nn_DataEmbedding (embedding_lookup).

Reference computation (B=32, L=4096, C_IN=7, D=512):
  out = value_emb + pos_emb + temp_emb
  value_emb = TokenEmbedding(x) @ proj_w.T + proj_b   (73+1 tiny conv1d's, k=8)
  pos_emb   = sinusoid_table(L, D)
  temp_emb  = sum of 4 fixed sinusoid-table lookups from x_mark (indices in [0,7))

Device algorithm (per core, 4 batches):
  * TokenEmbedding+projection collapse into ONE size-8 conv over L:
      value_emb[b,l,d] = sum_{m,c} A[d,m,c] * xpad[b, l+m, c] + proj_b[d]
    with A = einsum(proj_w[:, :511].reshape(D,73,7), kernels[:73]) + c==0 term.
  * The 4 temporal lookups are a 28-row one-hot matmul (tables only ever
    indexed at rows 0..6 where all four sinusoid tables agree).
  * The whole stationary operand (one-hot rows stacked over im2col rows)
    is built host-side — a pure relayout of x / x_mark — so each batch
    needs exactly ONE input DMA.  One K=84 bf16 matmul per 128-position
    tile does all the math; pos_emb + proj_b are one [L, D] bf16 table
    added during PSUM eviction.

Performance structure (PE-bound: LDW+MM pairs measure 677ns sustained
on this part, 128 pairs ~= 87us/core floor; everything else sits under):
  * all-bf16 (tolerance 2e-2, actual err ~6e-3).
  * positions interleaved stride-8 within each 1024-position group (the
    interleave is applied host-side so device APs are contiguous):
    PSUM partition p always holds 8 consecutive output rows -> every
    output DMA is one fully contiguous 1MB transfer (128 x 8KB), pos
    loads likewise.
  * PSUM eviction at 2-bank granularity (FD=1024) to amortize per-op
    overhead and halve semaphore traffic: per 4 pairs, 2 go DVE-direct
    (tensor_tensor add from PSUM), 2 go ScE-copy + DVE bf16 add (2x).
  * output DMAs dispatch on the Act HWDGE ring, inputs on SP, so input
    dispatches never queue behind 1MB output transfers.

Sharding: pure data parallel over batch: 32 batches -> 8 cores x 4 batches.
"""

import os
import sys
import ml_dtypes
import numpy as np

for _p in ("/opt/trn_rl_repo", "/opt/pypackages"):
    if os.path.isdir(_p) and _p not in sys.path:
        sys.path.append(_p)

from contextlib import ExitStack

import concourse.bass as bass
import concourse.tile as tile
from concourse import bacc, mybir
from concourse.bass_utils import run_bass_kernel_spmd

# ---------------------------------------------------------------- constants
B, L, C_IN, D = 32, 4096, 7, 512
KS, NK, M = 8, 74, 7          # kernel_size, num_kernels, history
PROJ_IN = 73 * C_IN + 1       # 512
N_CORES = 8
NB = B // N_CORES             # batches per core = 4
KIM = KS * C_IN               # im2col rows = 56
KOH = 4 * 7                   # one-hot rows = 28
KTOT = KIM + KOH              # fused contraction = 84
P = 128                       # positions per matmul tile
GT = 8                        # tiles per group (position stride)
G = P * GT                    # positions per group = 1024
NG = L // G                   # groups per batch = 4
PAIR = 2 * D                  # eviction unit: 2 PSUM banks = 1024

F32 = mybir.dt.float32
BF16 = mybir.dt.bfloat16


def _sinusoid_table(n, d):
    pos = np.arange(n, dtype=np.float32)[:, None]
    div = np.exp(np.arange(0, d, 2, dtype=np.float32) * (-np.log(10000.0) / d))
    tab = np.zeros((n, d), dtype=np.float32)
    tab[:, 0::2] = np.sin(pos * div)
    tab[:, 1::2] = np.cos(pos * div)
    return tab


_POS_CACHE = None


def _pos_const():
    global _POS_CACHE
    if _POS_CACHE is None:
        _POS_CACHE = _sinusoid_table(L, D)
    return _POS_CACHE


# column permutation: device matmul tile (g, t) takes columns
# [ (g*GT + t)*P : +P ] of the permuted operand; column (g,t,p) must hold
# position g*G + 8p + t so that PSUM partition p = output row g*G+8p+t.
_PERM = None


def _col_perm():
    global _PERM
    if _PERM is None:
        l = np.arange(L)
        g, r = l // G, l % G
        t, p = r // P, r % P
        _PERM = (g * G + p * GT + t).astype(np.int64)  # perm[j'] = source pos
    return _PERM


def _host_prep(x, x_mark, kernels, proj_w, proj_b):
    """Build per-core inputs. All heavy math stays on device; this is layout
    glue plus the tiny [512,511]x[73,8] weight fold."""
    x = np.asarray(x, dtype=np.float32)
    x_mark = np.asarray(x_mark)
    kernels = np.asarray(kernels, dtype=np.float32)
    proj_w = np.asarray(proj_w, dtype=np.float32)
    proj_b = np.asarray(proj_b, dtype=np.float32)

    # full stationary operand [B, 84, L] bf16 (pure relayout of x/x_mark):
    #   rows 0..27   one-hot: row 7j+v = (x_mark[:, :, j] == v)
    #   rows 28..83  im2col:  row 28+7m+c = xpad[:, c, l+m]
    lhs = np.empty((B, KTOT, L), dtype=ml_dtypes.bfloat16)
    xm = x_mark.astype(np.int64)
    oh = xm[:, :, :, None] == np.arange(7)[None, None, None, :]   # [B,L,4,7]
    lhs[:, :KOH, :] = oh.transpose(0, 2, 3, 1).reshape(B, KOH, L)
    xpad = np.zeros((B, C_IN, L + KS), dtype=np.float32)
    xpad[:, :, M : M + L] = x.transpose(0, 2, 1)
    for m in range(KS):
        lhs[:, KOH + C_IN * m : KOH + C_IN * (m + 1), :] = xpad[
            :, :, m : m + L
        ]
    # stride-8 position interleave (see _col_perm)
    lhs = np.ascontiguousarray(lhs[:, :, _col_perm()])

    # fused conv weight A[d, m, c]
    p3 = proj_w[:, : 73 * C_IN].reshape(D, 73, C_IN)
    A = np.einsum("dkc,km->dmc", p3, kernels[:73], dtype=np.float32)
    A[:, :, 0] += np.outer(proj_w[:, 511], kernels[73])
    w_pack = A.transpose(1, 2, 0).reshape(KIM, D)  # row 7m+c

    # temporal tables: all four sinusoid tables agree on rows 0..6.
    tab7 = _sinusoid_table(7, D)  # [7, D]
    wtab = np.concatenate([np.tile(tab7, (4, 1)), w_pack], axis=0)  # [84, D]
    wtab = np.ascontiguousarray(wtab.astype(ml_dtypes.bfloat16))

    # positional + bias table (bf16: |values| <= ~1, rounding ~2e-3 abs,
    # negligible vs output scale ~22), rows in interleaved order so the
    # SBUF tile [128, NG*GT*D] has partition p = rows {g*G+8p+t}.
    posb = (_pos_const() + proj_b[None, :]).astype(ml_dtypes.bfloat16)
    # row r of interleaved table = position g*G + 8p + t where the SBUF
    # flat index is ((p * NG) + g) * GT + t ... simpler: build per-partition
    # layout directly: part p, free [g, t, d] = posb[g*G + 8p + t, d]
    pos_il = posb.reshape(NG, P, GT, D)            # [g, p, t, d]
    pos_il = np.ascontiguousarray(
        pos_il.transpose(1, 0, 2, 3).reshape(P, NG * GT * D)
    )  # [p, (g t d)]

    in_maps = []
    for core in range(N_CORES):
        sl = slice(core * NB, (core + 1) * NB)
        in_maps.append(
            {
                "lhs": np.ascontiguousarray(lhs[sl]),
                "wtab": wtab,
                "posil": pos_il,
            }
        )
    return in_maps


# ---------------------------------------------------------------- bass build
def build_nc(stage_bufs=6):
    nc = bacc.Bacc("TRN2", target_bir_lowering=False, debug=False)

    lhs_d = nc.dram_tensor("lhs", (NB, KTOT, L), BF16, kind="ExternalInput")
    wtab_d = nc.dram_tensor("wtab", (KTOT, D), BF16, kind="ExternalInput")
    posil_d = nc.dram_tensor("posil", (P, NG * GT * D), BF16,
                             kind="ExternalInput")
    out_d = nc.dram_tensor("out", (NB, L, D), BF16, kind="ExternalOutput")

    with tile.TileContext(nc) as tc, ExitStack() as ctx:
        dma = nc.sync        # input DMAs: SP HWDGE ring
        odma = nc.scalar     # output DMAs: Act HWDGE ring
        consts = ctx.enter_context(tc.tile_pool(name="consts", bufs=1))
        lhs_pool = ctx.enter_context(tc.tile_pool(name="lhsp", bufs=2))
        stage_pool = ctx.enter_context(tc.tile_pool(name="stage", bufs=stage_bufs))
        psum_pool = ctx.enter_context(
            tc.tile_pool(name="psum", bufs=4, space="PSUM")
        )

        # batch-0 operand first, in group-sized chunks so the first matmul
        # can start after ~170KB instead of the full 688KB
        lhs0 = lhs_pool.tile([KTOT, L], BF16, tag="lhs", name="lhs0")
        for g in range(NG):
            dma.dma_start(
                lhs0[:, g * G : (g + 1) * G], lhs_d.ap()[0, :, g * G : (g + 1) * G]
            )

        wtab_s = consts.tile([KTOT, D], BF16, tag="wtab")
        dma.dma_start(wtab_s[:], wtab_d.ap())

        # positional(+bias) table, one DMA, SBUF-resident [128, NG*GT*D]
        pos_s = consts.tile([P, NG * GT * D], BF16, tag="pos")
        dma.dma_start(pos_s[:], posil_d.ap())

        lhs = lhs0
        for b in range(NB):
            if b + 1 < NB:
                next_lhs = lhs_pool.tile([KTOT, L], BF16, tag="lhs",
                                         name=f"lhs{b + 1}")
                dma.dma_start(next_lhs[:], lhs_d.ap()[b + 1])
            else:
                next_lhs = None
            for g in range(NG):
                stage = stage_pool.tile([P, GT * D], BF16, tag="stage")
                for j in range(4):          # 4 pair-units of 2 tiles
                    ps = psum_pool.tile([P, PAIR], F32, tag="ps")
                    for h in range(2):
                        t = 2 * j + h
                        nc.tensor.matmul(
                            ps[:, D * h : D * (h + 1)],
                            lhs[:, (g * GT + t) * P : (g * GT + t + 1) * P],
                            wtab_s[:],
                            start=True,
                            stop=True,
                        )
                    ssl = slice(PAIR * j, PAIR * (j + 1))
                    psl = slice((g * GT + 2 * j) * D, (g * GT + 2 * j + 2) * D)
                    if j % 2 == 0:
                        # DVE: add pos straight out of PSUM (1x, FD=1024)
                        nc.vector.tensor_tensor(
                            out=stage[:, ssl],
                            in0=ps[:],
                            in1=pos_s[:, psl],
                            op=mybir.AluOpType.add,
                        )
                    else:
                        # ScE copy PSUM->SBUF, then DVE bf16 add (2x)
                        nc.scalar.copy(stage[:, ssl], ps[:])
                        nc.vector.tensor_tensor(
                            out=stage[:, ssl],
                            in0=stage[:, ssl],
                            in1=pos_s[:, psl],
                            op=mybir.AluOpType.add,
                        )
                # partition p holds output rows g*G + 8p .. g*G + 8p + 7:
                # one fully contiguous 1MB transfer, 128 x 8KB descriptors.
                # Alternate between the two HWDGE rings (SP carries little
                # after startup) so output transfers drain in parallel.
                dst = out_d.ap()[b, g * G : (g + 1) * G, :]
                dst = dst.rearrange("(p t) d -> p (t d)", p=P)
                (odma if g % 2 == 0 else dma).dma_start(dst, stage[:])
            lhs = next_lhs

    nc.compile()
    return nc


_NC_CACHE = None


def _get_nc():
    global _NC_CACHE
    if _NC_CACHE is None:
        _NC_CACHE = build_nc()
    return _NC_CACHE


TRACE = False          # set by test.py to capture an NTFF profile
LAST_RESULT = None     # BassKernelResults of the most recent run


def _run_once(in_maps):
    global LAST_RESULT
    nc = _get_nc()
    res = run_bass_kernel_spmd(
        nc, in_maps, core_ids=list(range(N_CORES)), trace=TRACE
    )
    LAST_RESULT = res
    return np.concatenate(
        [np.asarray(r["out"], dtype=np.float32) for r in res.results], axis=0
    )


def _run_subprocess(inputs):
    """Crash-isolated fallback: run in a fresh interpreter (a device fault can
    wedge the parent process's jax runtime)."""
    import pickle
    import subprocess
    import tempfile

    with tempfile.TemporaryDirectory() as td:
        fin = os.path.join(td, "in.pkl")
        fout = os.path.join(td, "out.npy")
        with open(fin, "wb") as f:
            pickle.dump(inputs, f)
        code = (
            "import pickle, numpy as np, sys;"
            f"sys.path.insert(0, {os.path.dirname(os.path.abspath(__file__))!r});"
            "import kernel as K;"
            f"ins = pickle.load(open({fin!r}, 'rb'));"
            "out = K._run_once(K._host_prep(**ins));"
            f"np.save({fout!r}, out)"
        )
        subprocess.run([sys.executable, "-c", code], check=True, timeout=1800)
        return np.load(fout)


def kernel(x, x_mark, kernels, proj_w, proj_b):
    inputs = dict(x=x, x_mark=x_mark, kernels=kernels, proj_w=proj_w,
                  proj_b=proj_b)
    in_maps = _host_prep(**inputs)
    # the TRN fleet shows rare transient NRT_EXEC_UNIT_UNRECOVERABLE faults;
    # retry in-process first, then in fresh subprocesses.
    for attempt in range(2):
        try:
            return _run_once(in_maps)
        except Exception:
            pass
    for attempt in range(3):
        try:
            return _run_subprocess(inputs)
        except Exception:
            if attempt == 2:
                raise
    raise RuntimeError("unreachable")
